# revision 14
# baseline (speedup 1.0000x reference)
"""Trainium2 Bass kernel for nn_BlurF: depthwise 4x4 blur (upfirdn2d pad=(2,1)).

Strategy: data-parallel over batch (8 cores x 1 image of [128,256,256]).
Per core, the separable conv is computed as two PE banded-matmul passes
using the data as the stationary operand, which transposes each pass:
  pass1: VT[x, y'] = sum_y X[y, x] * Bv[y, y']   (vertical conv, transposed)
  pass2: OUT[y', x'] = sum_x VT[x, y'] * Bh[x, x'] (horizontal conv, back)
Boundary zero-padding is folded into the band matrices.

PE-work reduction: the band is 4 taps wide, so the y<128 stationary tile
only contributes to output columns [0,130) and the y>=128 tile only to
[127,256). Each stationary streams just its live column range; the
3-column overlap accumulates via PSUM per-element has_written bits
(first matmul of a group start=True clears the whole bank, later
matmuls overwrite-where-unwritten / accumulate-where-written). This
halves streamed PE columns vs streaming all 256 on both tiles, with
identical math (the dropped band columns are exactly zero).

Both m-halves of pass1 (and q-halves of pass2) share one [128,512] f32
PSUM bank, so each j needs a single PSUM->SBUF copy per pass,
alternated between DVE and ACT. All DMAs issue from SP (sync).

i8 mode (default): input is host-quantized to int8 (s_in = absmax/127,
no clipping) and cast int8->fp16 during the input DMA (SWDGE); matmuls
stay fp16 with f32 PSUM; the pass-2 PSUM->SBUF copy converts f32->int8
(RNE, saturating) with s_in/s_out folded into the horizontal bands; the
output DMA moves int8. Host dequantizes by s_out. This halves DMA bytes
vs fp16io; rel L2 err ~1.5e-2 from the two quantizations (gate 2e-2).
General non-separable 4x4 kernels still work via SVD.
"""

import numpy as np
import concourse.bacc as bacc
import concourse.mybir as mybir
from concourse.tile import TileContext
from concourse.bass_utils import run_bass_kernel_spmd

N_CORES = 8
C, H, W = 128, 256, 256
PRECISION = "i8"  # int8 in+out (~1.5e-2) | "fp16io" (~3e-4) | "fp16" | "fp32r" | "fp32"
# Output quantization scale: s_out = OUT_RANGE * sigma_in * ||k||_F / 127.
# 5.2 covers the output absmax of ~33M-sample gaussian shards with ~0.5%
# headroom (no clipping); a saturation-count retry in kernel() guards the
# general case.
OUT_RANGE = 5.2

_BUILD_CACHE = {}


def _round_f32r(a):
    """Round fp32 array to float32r (11 stored mantissa bits), round-half-up."""
    b = np.ascontiguousarray(a, dtype=np.float32).view(np.uint32)
    b = (b + np.uint32(0x800)) & np.uint32(0xFFFFF000)
    return b.view(np.float32)


def _factorize(kernel4x4):
    """kernel[a,b] = sum_r u_r[a] v_r[b]; returns list of (u, v) float64."""
    k = np.asarray(kernel4x4, dtype=np.float64)
    U, S, Vt = np.linalg.svd(k)
    comps = []
    for r in range(4):
        if S[r] > 1e-9 * max(S[0], 1e-30):
            comps.append((U[:, r] * np.sqrt(S[r]), Vt[r, :] * np.sqrt(S[r])))
    return comps


def _band(taps, n):
    """B[s, s'] = taps[a] where s' = s + a - 1, for a in 0..3, clipped to [0,n)."""
    B = np.zeros((n, n), dtype=np.float64)
    for a in range(4):
        lo = max(0, 1 - a)
        hi = min(n, n + 1 - a)
        s = np.arange(lo, hi)
        B[s, s + a - 1] = taps[a]
    return B


# Live band column ranges per 128-row stationary half (taps span y' in
# [y-1, y+2]): half 0 (rows 0..127) -> cols [0,130); half 1 (rows
# 128..255) -> cols [127,256).
T0_HI = 130
T1_LO = 127

DEFAULT_CFG = dict(G=16, ds=2, pd=3, cb=1, qin="gpsimd", qout="sync",
                   ycs=0, xin_bufs=2, vt_bufs=8, yout_bufs=3,
                   p1_bufs=4, p2_bufs=4, split=0)


def _emit(nc, tc, x, y, bvt, bht, rank, precision, cfg=None):
    cfg = {**DEFAULT_CFG, **(cfg or {})}
    Gc = cfg["G"]
    ds = cfg["ds"]
    cb = cfg["cb"]  # copy batch: j's per PSUM tile / PSUM->SBUF copy
    gsz = Gc // ds
    f32 = mybir.dt.float32
    mmdt = {"fp32": f32, "fp16": mybir.dt.float16, "i8": mybir.dt.float16,
            "fp16io": mybir.dt.float16}.get(precision, mybir.dt.float32r)
    ydt = {"fp16io": mybir.dt.float16, "i8": mybir.dt.int8}.get(precision, f32)
    NG = C // Gc
    # int8 input is cast to fp16 during the DMA -> SWDGE (gpsimd) required
    qin = nc.gpsimd if precision == "i8" else getattr(nc, cfg["qin"])
    qout = getattr(nc, cfg["qout"])
    with (
        tc.tile_pool(name="xin", bufs=cfg["xin_bufs"]) as xin_pool,
        tc.tile_pool(name="vt", bufs=cfg["vt_bufs"]) as vt_pool,
        tc.tile_pool(name="yout", bufs=cfg["yout_bufs"]) as yout_pool,
        tc.tile_pool(name="p1", bufs=cfg["p1_bufs"], space="PSUM") as p1_pool,
        tc.tile_pool(name="p2", bufs=cfg["p2_bufs"], space="PSUM") as p2_pool,
    ):
        pending = []

        def copy_engine(i, which):
            # alternate DVE/ACT; 'which' flips so pass1/pass2 copies of the
            # same batch land on different engines
            if (i + which) % 2 == 0:
                return nc.vector.tensor_copy
            return lambda o, s: nc.scalar.copy(o, s)

        def emit_pass2(p):
            vt, yout, jj, g = p
            p2 = p2_pool.tile([128, 512 * cb], f32, tag="p2")
            for dj in range(cb):
                for q in (0, 1):
                    ops = []
                    for r in range(rank):
                        ops.append((vt[:, dj * 512 + q * 128:
                                       dj * 512 + (q + 1) * 128],
                                    bht[r][0][:, 0:T0_HI], 0, T0_HI))
                        ops.append((vt[:, dj * 512 + 256 + q * 128:
                                       dj * 512 + 256 + (q + 1) * 128],
                                    bht[r][1][:, T1_LO:256], T1_LO, 256))
                    for i, (lhsT, rhs, lo, hi) in enumerate(ops):
                        nc.tensor.matmul(
                            p2[:, dj * 512 + q * 256 + lo:
                               dj * 512 + q * 256 + hi], lhsT, rhs,
                            start=(i == 0), stop=(i == len(ops) - 1),
                        )
            if cfg["ycs"]:
                # fixed engine per q-half so each output DMA chunk waits
                # on exactly one copy engine's stream
                nc.vector.tensor_copy(
                    yout[:, jj:jj + cb, 0:256],
                    p2[:].rearrange("p (j x) -> p j x", x=512)[:, :, 0:256]
                    if cb > 1 else p2[:, 0:256])
                nc.scalar.copy(
                    yout[:, jj:jj + cb, 256:512],
                    p2[:].rearrange("p (j x) -> p j x", x=512)[:, :, 256:512]
                    if cb > 1 else p2[:, 256:512])
            else:
                copy_engine(jj // cb, 1)(yout[:, jj:jj + cb, :], p2[:])
            if (jj + cb) % gsz == 0:
                h = (jj + cb) // gsz - 1
                c0 = g * Gc + h * gsz
                for q in (0, 1):
                    # y DRAM layout is [y', c, x']: contiguous (c,x) runs
                    # of gsz*512 bytes per partition row
                    qout.dma_start(
                        out=y[q * 128:(q + 1) * 128, c0:c0 + gsz, :],
                        in_=yout[:, h * gsz:(h + 1) * gsz,
                                 q * 256:(q + 1) * 256],
                    )

        for g in range(NG):
            xins = []
            for t in (0, 1):
                xt = xin_pool.tile([128, Gc, 256], mmdt,
                                   tag=f"xin{t}", name=f"xin{t}")
                if cfg["split"] and t == 1:
                    # raw int8 load on HWDGE + int8->fp16 convert on gpsimd:
                    # halves this tile's DMA-fabric SBUF writes and keeps
                    # the cast off the (busy) DVE/ACT engines
                    x8t = xin_pool.tile([128, Gc, 256], mybir.dt.int8,
                                        tag=f"x8{t}", name=f"x8{t}")
                    nc.sync.dma_start(
                        out=x8t[:],
                        in_=x[t * 128:(t + 1) * 128, g * Gc:(g + 1) * Gc, :],
                    )
                    nc.gpsimd.tensor_copy(xt[:], x8t[:])
                else:
                    # x DRAM layout is [y, c, x]: contiguous runs per row
                    qin.dma_start(
                        out=xt[:],
                        in_=x[t * 128:(t + 1) * 128, g * Gc:(g + 1) * Gc, :],
                    )
                xins.append(xt)
            yout = yout_pool.tile([128, Gc, 512], ydt, tag="yout", name="yout")
            for jj in range(0, Gc, cb):
                p1 = p1_pool.tile([128, 512 * cb], f32, tag="p1")
                for dj in range(cb):
                    j = jj + dj
                    for m in (0, 1):
                        ops = []
                        for r in range(rank):
                            ops.append((xins[0][:, j, m * 128:(m + 1) * 128],
                                        bvt[r][0][:, 0:T0_HI], 0, T0_HI))
                            ops.append((xins[1][:, j, m * 128:(m + 1) * 128],
                                        bvt[r][1][:, T1_LO:256], T1_LO, 256))
                        for i, (lhsT, rhs, lo, hi) in enumerate(ops):
                            nc.tensor.matmul(
                                p1[:, dj * 512 + m * 256 + lo:
                                   dj * 512 + m * 256 + hi], lhsT, rhs,
                                start=(i == 0), stop=(i == len(ops) - 1),
                            )
                vt = vt_pool.tile([128, 512 * cb], mmdt, tag="vt", name="vt")
                copy_engine(jj // cb, 0)(vt[:], p1[:])
                pending.append((vt, yout, jj, g))
                if len(pending) > cfg["pd"]:
                    emit_pass2(pending.pop(0))
        for p in pending:
            emit_pass2(p)


def _build(rank, precision, reps=1, loop_reps=None, cfg=None):
    key = (rank, precision, reps, loop_reps,
           tuple(sorted((cfg or {}).items())))
    if key in _BUILD_CACHE:
        return _BUILD_CACHE[key]
    f32 = mybir.dt.float32
    mmdt = {"fp32": f32, "fp16": mybir.dt.float16, "i8": mybir.dt.float16,
            "fp16io": mybir.dt.float16}.get(precision, mybir.dt.float32r)
    xdt = {"fp32": f32, "i8": mybir.dt.int8}.get(precision, mmdt)
    ydt = {"fp16io": mybir.dt.float16, "i8": mybir.dt.int8}.get(precision, f32)
    nc = bacc.Bacc("TRN2", target_bir_lowering=False, debug=False)
    x = nc.dram_tensor("x", [H, C, W], xdt, kind="ExternalInput").ap()
    bv = nc.dram_tensor("bv", [rank, 2, 128, 256], mmdt, kind="ExternalInput").ap()
    bh = nc.dram_tensor("bh", [rank, 2, 128, 256], mmdt, kind="ExternalInput").ap()
    y = nc.dram_tensor("y", [H, C, W], ydt, kind="ExternalOutput").ap()
    with TileContext(nc) as tc:
        with tc.tile_pool(name="bands", bufs=1) as band_pool:
            bvt = [[None, None] for _ in range(rank)]
            bht = [[None, None] for _ in range(rank)]
            for r in range(rank):
                for t in (0, 1):
                    bvt[r][t] = band_pool.tile([128, 256], mmdt, tag=f"bv{r}{t}", name=f"bv{r}{t}")
                    nc.sync.dma_start(out=bvt[r][t][:], in_=bv[r, t])
                    bht[r][t] = band_pool.tile([128, 256], mmdt, tag=f"bh{r}{t}", name=f"bh{r}{t}")
                    nc.sync.dma_start(out=bht[r][t][:], in_=bh[r, t])
            if loop_reps is not None:
                with tc.For_i(0, loop_reps, 1):
                    _emit(nc, tc, x, y, bvt, bht, rank, precision, cfg)
            else:
                for _ in range(reps):
                    _emit(nc, tc, x, y, bvt, bht, rank, precision, cfg)
    nc.compile()
    _BUILD_CACHE[key] = nc
    return nc


def _prep_inputs(fmap, kernel4x4, precision, s_out_boost=1.0):
    comps = _factorize(kernel4x4)
    rank = max(1, len(comps))
    while len(comps) < rank:
        comps.append((np.zeros(4), np.zeros(4)))

    def bands(comps_i):
        bv = np.zeros((rank, 2, 128, 256), dtype=np.float32)
        bh = np.zeros((rank, 2, 128, 256), dtype=np.float32)
        for r, (u, v) in enumerate(comps_i):
            bv[r] = _band(u, H).astype(np.float32).reshape(2, 128, 256)
            bh[r] = _band(v, W).astype(np.float32).reshape(2, 128, 256)
        if precision == "fp32r":
            return _round_f32r(bv), _round_f32r(bh)
        if precision in ("fp16", "fp16io", "i8"):
            return bv.astype(np.float16), bh.astype(np.float16)
        return bv, bh

    knorm = float(np.sqrt(np.square(np.asarray(kernel4x4, np.float64)).sum()))
    in_maps, s_outs = [], []
    if precision != "i8":
        bv, bh = bands(comps)
    for i in range(N_CORES):
        shard = np.ascontiguousarray(fmap[i].transpose(1, 0, 2),
                                     dtype=np.float32)  # [y, c, x]
        if precision == "fp32r":
            shard = _round_f32r(shard)
        elif precision in ("fp16", "fp16io"):
            shard = shard.astype(np.float16)
        elif precision == "i8":
            s_in = float(np.abs(shard).max()) / 127.0
            s_out = OUT_RANGE * float(shard.std()) * knorm / 127.0 * s_out_boost
            s_outs.append(s_out)
            alpha = s_in / s_out
            bv, bh = bands([(u, v * alpha) for (u, v) in comps])
            shard = np.clip(np.rint(shard * (1.0 / s_in)),
                            -127, 127).astype(np.int8)
        in_maps.append({"x": shard, "bv": bv, "bh": bh})
    return rank, in_maps, s_outs


def _run(nc, in_maps):
    last_err = None
    for _attempt in range(3):
        try:
            return run_bass_kernel_spmd(nc, in_maps, list(range(N_CORES)),
                                        trace=False)
        except Exception as e:  # transient device wedge -> retry
            last_err = e
            import time
            time.sleep(2.0)
    raise last_err


def kernel(fmap, kernel):
    fmap = np.asarray(fmap)
    kern = np.asarray(kernel)
    assert fmap.shape == (N_CORES, C, H, W), fmap.shape
    boost = 1.0
    rank, in_maps, s_outs = _prep_inputs(fmap, kern, PRECISION)
    nc = _build(rank, PRECISION)
    res = _run(nc, in_maps)
    if PRECISION == "i8":
        # s_out underestimated the output range -> saturation; retry coarser
        for _ in range(3):
            n_sat = sum(int((res.results[i]["y"] == 127).sum() +
                            (res.results[i]["y"] == -128).sum())
                        for i in range(N_CORES))
            if n_sat <= 4096:
                break
            boost *= 1.5
            rank, in_maps, s_outs = _prep_inputs(fmap, kern, PRECISION,
                                                 s_out_boost=boost)
            res = _run(nc, in_maps)
    out = np.stack([res.results[i]["y"].transpose(1, 0, 2)
                    for i in range(N_CORES)], axis=0)
    out = out.astype(np.float32)
    if PRECISION == "i8":
        out *= np.asarray(s_outs, np.float32)[:, None, None, None]
    return np.ascontiguousarray(out)



# revision 25
# speedup vs baseline: 1.0038x; 1.0038x over previous
"""Trainium2 Bass kernel for nn_BlurF: depthwise 4x4 blur (upfirdn2d pad=(2,1)).

Strategy: data-parallel over batch (8 cores x 1 image of [128,256,256]).
Per core, the separable conv is computed as two PE banded-matmul passes
using the data as the stationary operand, which transposes each pass:
  pass1: VT[x, y'] = sum_y X[y, x] * Bv[y, y']   (vertical conv, transposed)
  pass2: OUT[y', x'] = sum_x VT[x, y'] * Bh[x, x'] (horizontal conv, back)
Boundary zero-padding is folded into the band matrices.

PE-work reduction: the band is 4 taps wide, so the y<128 stationary tile
only contributes to output columns [0,130) and the y>=128 tile only to
[127,256). Each stationary streams just its live column range; the
3-column overlap accumulates via PSUM per-element has_written bits
(first matmul of a group start=True clears the whole bank, later
matmuls overwrite-where-unwritten / accumulate-where-written). This
halves streamed PE columns vs streaming all 256 on both tiles, with
identical math (the dropped band columns are exactly zero).

Both m-halves of pass1 (and q-halves of pass2) share one [128,512] f32
PSUM bank, so each j needs a single PSUM->SBUF copy per pass,
alternated between DVE and ACT. All DMAs issue from SP (sync).

i8 mode (default): input is host-quantized to int8 (s_in = absmax/127,
no clipping) and cast int8->fp16 during the input DMA (SWDGE); matmuls
stay fp16 with f32 PSUM; the pass-2 PSUM->SBUF copy converts f32->int8
(RNE, saturating) with s_in/s_out folded into the horizontal bands; the
output DMA moves int8. Host dequantizes by s_out. This halves DMA bytes
vs fp16io; rel L2 err ~1.5e-2 from the two quantizations (gate 2e-2).
General non-separable 4x4 kernels still work via SVD.
"""

import numpy as np
import concourse.bacc as bacc
import concourse.mybir as mybir
from concourse.tile import TileContext
from concourse.bass_utils import run_bass_kernel_spmd

N_CORES = 8
C, H, W = 128, 256, 256
PRECISION = "i8"  # int8 in+out (~1.5e-2) | "fp16io" (~3e-4) | "fp16" | "fp32r" | "fp32"
# Output quantization scale: s_out = OUT_RANGE * sigma_in * ||k||_F / 127.
# 5.2 covers the output absmax of ~33M-sample gaussian shards with ~0.5%
# headroom (no clipping); a saturation-count retry in kernel() guards the
# general case.
OUT_RANGE = 5.2

_BUILD_CACHE = {}


def _round_f32r(a):
    """Round fp32 array to float32r (11 stored mantissa bits), round-half-up."""
    b = np.ascontiguousarray(a, dtype=np.float32).view(np.uint32)
    b = (b + np.uint32(0x800)) & np.uint32(0xFFFFF000)
    return b.view(np.float32)


def _factorize(kernel4x4):
    """kernel[a,b] = sum_r u_r[a] v_r[b]; returns list of (u, v) float64."""
    k = np.asarray(kernel4x4, dtype=np.float64)
    U, S, Vt = np.linalg.svd(k)
    comps = []
    for r in range(4):
        if S[r] > 1e-9 * max(S[0], 1e-30):
            comps.append((U[:, r] * np.sqrt(S[r]), Vt[r, :] * np.sqrt(S[r])))
    return comps


def _band(taps, n):
    """B[s, s'] = taps[a] where s' = s + a - 1, for a in 0..3, clipped to [0,n)."""
    B = np.zeros((n, n), dtype=np.float64)
    for a in range(4):
        lo = max(0, 1 - a)
        hi = min(n, n + 1 - a)
        s = np.arange(lo, hi)
        B[s, s + a - 1] = taps[a]
    return B


# Live band column ranges per 128-row stationary half (taps span y' in
# [y-1, y+2]): half 0 (rows 0..127) -> cols [0,130); half 1 (rows
# 128..255) -> cols [127,256).
T0_HI = 130
T1_LO = 127

# n1p: channels per group routed through the single-pass 2D conv (one
# PSUM->SBUF copy per element instead of two, at 2x the PE streaming).
# Their output lands transposed [x', y']; the host transposes back.
# Must be < G//ds so each h-half's output DMA still fires from a 2-pass
# channel. 0 = all channels two-pass (pre-mixed behavior).
DEFAULT_CFG = dict(G=16, ds=2, pd=3, cb=1, qin="gpsimd", qout="sync",
                   ycs=0, xin_bufs=2, vt_bufs=6, yout_bufs=2,
                   p1_bufs=4, p2_bufs=4, split=0, n1p=0)


def _emit(nc, tc, x, y, bvt, bht, bq, rank, precision, cfg=None):
    cfg = {**DEFAULT_CFG, **(cfg or {})}
    Gc = cfg["G"]
    ds = cfg["ds"]
    cb = cfg["cb"]  # copy batch: j's per PSUM tile / PSUM->SBUF copy
    n1p = cfg["n1p"]
    assert n1p < Gc // ds
    # with 1-pass channels, each channel row is padded to 260 cols (2 zero
    # cols each side) so x-shifted stationary windows never leave the tile
    XP = 260 if n1p else 256
    xo = 2 if n1p else 0
    gsz = Gc // ds
    f32 = mybir.dt.float32
    mmdt = {"fp32": f32, "fp16": mybir.dt.float16, "i8": mybir.dt.float16,
            "fp16io": mybir.dt.float16}.get(precision, mybir.dt.float32r)
    ydt = {"fp16io": mybir.dt.float16, "i8": mybir.dt.int8}.get(precision, f32)
    NG = C // Gc
    # int8 input is cast to fp16 during the DMA -> SWDGE (gpsimd) required
    qin = nc.gpsimd if precision == "i8" else getattr(nc, cfg["qin"])
    qout = getattr(nc, cfg["qout"])
    with (
        tc.tile_pool(name="xin", bufs=cfg["xin_bufs"]) as xin_pool,
        tc.tile_pool(name="vt", bufs=cfg["vt_bufs"]) as vt_pool,
        tc.tile_pool(name="yout", bufs=cfg["yout_bufs"]) as yout_pool,
        tc.tile_pool(name="p1", bufs=cfg["p1_bufs"], space="PSUM") as p1_pool,
        tc.tile_pool(name="p2", bufs=cfg["p2_bufs"], space="PSUM") as p2_pool,
    ):
        pending = []

        def copy_engine(i, which):
            # alternate DVE/ACT; 'which' flips so pass1/pass2 copies of the
            # same batch land on different engines
            if (i + which) % 2 == 0:
                return nc.vector.tensor_copy
            return lambda o, s: nc.scalar.copy(o, s)

        def emit_pass2(p):
            vt, yout, jj, g = p
            p2 = p2_pool.tile([128, 512 * cb], f32, tag="p2")
            for dj in range(cb):
                for q in (0, 1):
                    ops = []
                    for r in range(rank):
                        ops.append((vt[:, dj * 512 + q * 128:
                                       dj * 512 + (q + 1) * 128],
                                    bht[r][0][:, 0:T0_HI], 0, T0_HI))
                        ops.append((vt[:, dj * 512 + 256 + q * 128:
                                       dj * 512 + 256 + (q + 1) * 128],
                                    bht[r][1][:, T1_LO:256], T1_LO, 256))
                    for i, (lhsT, rhs, lo, hi) in enumerate(ops):
                        nc.tensor.matmul(
                            p2[:, dj * 512 + q * 256 + lo:
                               dj * 512 + q * 256 + hi], lhsT, rhs,
                            start=(i == 0), stop=(i == len(ops) - 1),
                        )
            if cfg["ycs"]:
                # fixed engine per q-half so each output DMA chunk waits
                # on exactly one copy engine's stream
                nc.vector.tensor_copy(
                    yout[:, jj:jj + cb, 0:256],
                    p2[:].rearrange("p (j x) -> p j x", x=512)[:, :, 0:256]
                    if cb > 1 else p2[:, 0:256])
                nc.scalar.copy(
                    yout[:, jj:jj + cb, 256:512],
                    p2[:].rearrange("p (j x) -> p j x", x=512)[:, :, 256:512]
                    if cb > 1 else p2[:, 256:512])
            else:
                copy_engine(jj // cb, 1)(yout[:, jj:jj + cb, :], p2[:])
            if (jj + cb) % gsz == 0:
                h = (jj + cb) // gsz - 1
                c0 = g * Gc + h * gsz
                for q in (0, 1):
                    # y DRAM layout is [y', c, x']: contiguous (c,x) runs
                    # of gsz*512 bytes per partition row
                    qout.dma_start(
                        out=y[q * 128:(q + 1) * 128, c0:c0 + gsz, :],
                        in_=yout[:, h * gsz:(h + 1) * gsz,
                                 q * 256:(q + 1) * 256],
                    )

        for g in range(NG):
            xins = []
            for t in (0, 1):
                xt = xin_pool.tile([128, Gc, XP], mmdt,
                                   tag=f"xin{t}", name=f"xin{t}")
                if cfg["split"] and t == 1:
                    # raw int8 load on HWDGE + int8->fp16 convert on gpsimd:
                    # halves this tile's DMA-fabric SBUF writes and keeps
                    # the cast off the (busy) DVE/ACT engines
                    x8t = xin_pool.tile([128, Gc, 256], mybir.dt.int8,
                                        tag=f"x8{t}", name=f"x8{t}")
                    nc.sync.dma_start(
                        out=x8t[:],
                        in_=x[t * 128:(t + 1) * 128, g * Gc:(g + 1) * Gc, :],
                    )
                    nc.gpsimd.tensor_copy(xt[:], x8t[:])
                else:
                    # x DRAM layout is [y, c, x]: contiguous runs per row
                    qin.dma_start(
                        out=xt[:, :, xo:xo + 256],
                        in_=x[t * 128:(t + 1) * 128, g * Gc:(g + 1) * Gc, :],
                    )
                if n1p and g < cfg["xin_bufs"]:
                    # zero the pad columns once per pool slot; later DMAs
                    # only write cols 2:258 so the pads stay zero
                    nc.gpsimd.memset(xt[:, :, 0:2], 0.0)
                    nc.gpsimd.memset(xt[:, :, 258:260], 0.0)
                xins.append(xt)
            yout = yout_pool.tile([128, Gc, 512], ydt, tag="yout", name="yout")
            for jj in range(0, Gc, cb):
                if jj < n1p:
                    assert cb == 1
                    # single-pass 2D conv: out^T[x', y'] accumulated from
                    # rank x 4 x-shifts x 2 y-halves stationaries
                    p = p1_pool.tile([128, 512], f32, tag="p1")
                    ops = []
                    for m in (0, 1):
                        for r in range(rank):
                            for b in range(4):
                                for t in (0, 1):
                                    lo, hi = (0, T0_HI) if t == 0 else (T1_LO, 256)
                                    ops.append((
                                        xins[t][:, jj,
                                                m * 128 + 3 - b:
                                                m * 128 + 131 - b],
                                        bq[r][b][t][:, lo:hi],
                                        m * 256 + lo, m * 256 + hi))
                    for i, (lhsT, rhs, o0, o1) in enumerate(ops):
                        nc.tensor.matmul(
                            p[:, o0:o1], lhsT, rhs,
                            start=(i == 0), stop=(i == len(ops) - 1),
                        )
                    copy_engine(jj, 0)(yout[:, jj:jj + 1, :], p[:])
                    continue
                p1 = p1_pool.tile([128, 512 * cb], f32, tag="p1")
                for dj in range(cb):
                    j = jj + dj
                    for m in (0, 1):
                        ops = []
                        for r in range(rank):
                            ops.append((xins[0][:, j, xo + m * 128:
                                                xo + (m + 1) * 128],
                                        bvt[r][0][:, 0:T0_HI], 0, T0_HI))
                            ops.append((xins[1][:, j, xo + m * 128:
                                                xo + (m + 1) * 128],
                                        bvt[r][1][:, T1_LO:256], T1_LO, 256))
                        for i, (lhsT, rhs, lo, hi) in enumerate(ops):
                            nc.tensor.matmul(
                                p1[:, dj * 512 + m * 256 + lo:
                                   dj * 512 + m * 256 + hi], lhsT, rhs,
                                start=(i == 0), stop=(i == len(ops) - 1),
                            )
                vt = vt_pool.tile([128, 512 * cb], mmdt, tag="vt", name="vt")
                copy_engine(jj // cb, 0)(vt[:], p1[:])
                pending.append((vt, yout, jj, g))
                if len(pending) > cfg["pd"]:
                    emit_pass2(pending.pop(0))
        for p in pending:
            emit_pass2(p)


def _build(rank, precision, reps=1, loop_reps=None, cfg=None):
    key = (rank, precision, reps, loop_reps,
           tuple(sorted((cfg or {}).items())))
    if key in _BUILD_CACHE:
        return _BUILD_CACHE[key]
    f32 = mybir.dt.float32
    mmdt = {"fp32": f32, "fp16": mybir.dt.float16, "i8": mybir.dt.float16,
            "fp16io": mybir.dt.float16}.get(precision, mybir.dt.float32r)
    xdt = {"fp32": f32, "i8": mybir.dt.int8}.get(precision, mmdt)
    ydt = {"fp16io": mybir.dt.float16, "i8": mybir.dt.int8}.get(precision, f32)
    n1p = ({**DEFAULT_CFG, **(cfg or {})})["n1p"]
    nc = bacc.Bacc("TRN2", target_bir_lowering=False, debug=False)
    x = nc.dram_tensor("x", [H, C, W], xdt, kind="ExternalInput").ap()
    bv = nc.dram_tensor("bv", [rank, 2, 128, 256], mmdt, kind="ExternalInput").ap()
    bh = nc.dram_tensor("bh", [rank, 2, 128, 256], mmdt, kind="ExternalInput").ap()
    if n1p:
        bqd = nc.dram_tensor("bq", [rank, 4, 2, 128, 256], mmdt,
                             kind="ExternalInput").ap()
    y = nc.dram_tensor("y", [H, C, W], ydt, kind="ExternalOutput").ap()
    with TileContext(nc) as tc:
        with tc.tile_pool(name="bands", bufs=1) as band_pool:
            bvt = [[None, None] for _ in range(rank)]
            bht = [[None, None] for _ in range(rank)]
            bq = [[[None, None] for _ in range(4)] for _ in range(rank)]
            for r in range(rank):
                for t in (0, 1):
                    bvt[r][t] = band_pool.tile([128, 256], mmdt, tag=f"bv{r}{t}", name=f"bv{r}{t}")
                    nc.sync.dma_start(out=bvt[r][t][:], in_=bv[r, t])
                    bht[r][t] = band_pool.tile([128, 256], mmdt, tag=f"bh{r}{t}", name=f"bh{r}{t}")
                    nc.sync.dma_start(out=bht[r][t][:], in_=bh[r, t])
                    if n1p:
                        for b in range(4):
                            bq[r][b][t] = band_pool.tile(
                                [128, 256], mmdt, tag=f"bq{r}{b}{t}",
                                name=f"bq{r}{b}{t}")
                            nc.sync.dma_start(out=bq[r][b][t][:],
                                              in_=bqd[r, b, t])
            if loop_reps is not None:
                with tc.For_i(0, loop_reps, 1):
                    _emit(nc, tc, x, y, bvt, bht, bq, rank, precision, cfg)
            else:
                for _ in range(reps):
                    _emit(nc, tc, x, y, bvt, bht, bq, rank, precision, cfg)
    nc.compile()
    _BUILD_CACHE[key] = nc
    return nc


def _prep_inputs(fmap, kernel4x4, precision, s_out_boost=1.0, n1p=None):
    if n1p is None:
        n1p = DEFAULT_CFG["n1p"]
    comps = _factorize(kernel4x4)
    rank = max(1, len(comps))
    while len(comps) < rank:
        comps.append((np.zeros(4), np.zeros(4)))

    def bands(comps_i):
        bv = np.zeros((rank, 2, 128, 256), dtype=np.float32)
        bh = np.zeros((rank, 2, 128, 256), dtype=np.float32)
        bq = np.zeros((rank, 4, 2, 128, 256), dtype=np.float32)
        for r, (u, v) in enumerate(comps_i):
            Bv = _band(u, H).astype(np.float32).reshape(2, 128, 256)
            bv[r] = Bv
            bh[r] = _band(v, W).astype(np.float32).reshape(2, 128, 256)
            for b in range(4):
                bq[r, b] = np.float32(v[b]) * Bv
        if precision == "fp32r":
            return _round_f32r(bv), _round_f32r(bh), _round_f32r(bq)
        if precision in ("fp16", "fp16io", "i8"):
            return (bv.astype(np.float16), bh.astype(np.float16),
                    bq.astype(np.float16))
        return bv, bh, bq

    knorm = float(np.sqrt(np.square(np.asarray(kernel4x4, np.float64)).sum()))
    in_maps, s_outs = [], []
    if precision != "i8":
        bv, bh, bq = bands(comps)
    for i in range(N_CORES):
        shard = np.ascontiguousarray(fmap[i].transpose(1, 0, 2),
                                     dtype=np.float32)  # [y, c, x]
        if precision == "fp32r":
            shard = _round_f32r(shard)
        elif precision in ("fp16", "fp16io"):
            shard = shard.astype(np.float16)
        elif precision == "i8":
            s_in = float(np.abs(shard).max()) / 127.0
            s_out = OUT_RANGE * float(shard.std()) * knorm / 127.0 * s_out_boost
            s_outs.append(s_out)
            alpha = s_in / s_out
            bv, bh, bq = bands([(u, v * alpha) for (u, v) in comps])
            shard = np.clip(np.rint(shard * (1.0 / s_in)),
                            -127, 127).astype(np.int8)
        m = {"x": shard, "bv": bv, "bh": bh}
        if n1p:
            m["bq"] = bq
        in_maps.append(m)
    return rank, in_maps, s_outs


def _run(nc, in_maps):
    last_err = None
    for _attempt in range(3):
        try:
            return run_bass_kernel_spmd(nc, in_maps, list(range(N_CORES)),
                                        trace=False)
        except Exception as e:  # transient device wedge -> retry
            last_err = e
            import time
            time.sleep(2.0)
    raise last_err


def kernel(fmap, kernel):
    fmap = np.asarray(fmap)
    kern = np.asarray(kernel)
    assert fmap.shape == (N_CORES, C, H, W), fmap.shape
    boost = 1.0
    rank, in_maps, s_outs = _prep_inputs(fmap, kern, PRECISION)
    nc = _build(rank, PRECISION)
    res = _run(nc, in_maps)
    if PRECISION == "i8":
        # s_out underestimated the output range -> saturation; retry coarser
        for _ in range(3):
            n_sat = sum(int((res.results[i]["y"] == 127).sum() +
                            (res.results[i]["y"] == -128).sum())
                        for i in range(N_CORES))
            if n_sat <= 4096:
                break
            boost *= 1.5
            rank, in_maps, s_outs = _prep_inputs(fmap, kern, PRECISION,
                                                 s_out_boost=boost)
            res = _run(nc, in_maps)
    out = np.stack([res.results[i]["y"].transpose(1, 0, 2)
                    for i in range(N_CORES)], axis=0)
    n1p, Gc = DEFAULT_CFG["n1p"], DEFAULT_CFG["G"]
    if n1p:
        # single-pass channels come back transposed [x', y']
        idx = [j for j in range(C) if j % Gc < n1p]
        out[:, idx] = np.swapaxes(out[:, idx], 2, 3)
    out = out.astype(np.float32)
    if PRECISION == "i8":
        out *= np.asarray(s_outs, np.float32)[:, None, None, None]
    return np.ascontiguousarray(out)



# revision 27
# speedup vs baseline: 1.1507x; 1.1463x over previous
"""Trainium2 Bass kernel for nn_BlurF: depthwise 4x4 blur (upfirdn2d pad=(2,1)).

Strategy: data-parallel over batch (8 cores x 1 image of [128,256,256]).
Per core, the separable conv is computed as two PE banded-matmul passes
using the data as the stationary operand, which transposes each pass:
  pass1: VT[x, y'] = sum_y X[y, x] * Bv[y, y']   (vertical conv, transposed)
  pass2: OUT[y', x'] = sum_x VT[x, y'] * Bh[x, x'] (horizontal conv, back)
Boundary zero-padding is folded into the band matrices.

PE-work reduction: the band is 4 taps wide, so the y<128 stationary tile
only contributes to output columns [0,130) and the y>=128 tile only to
[127,256). Each stationary streams just its live column range; the
3-column overlap accumulates via PSUM per-element has_written bits
(first matmul of a group start=True clears the whole bank, later
matmuls overwrite-where-unwritten / accumulate-where-written). This
halves streamed PE columns vs streaming all 256 on both tiles, with
identical math (the dropped band columns are exactly zero).

Both m-halves of pass1 (and q-halves of pass2) share one [128,512] f32
PSUM bank, so each j needs a single PSUM->SBUF copy per pass,
alternated between DVE and ACT. All DMAs issue from SP (sync).

i8 mode (default): input is host-quantized to int8 (s_in = absmax/127,
no clipping) and cast int8->fp16 during the input DMA (SWDGE); matmuls
stay fp16 with f32 PSUM; the pass-2 PSUM->SBUF copy converts f32->int8
(RNE, saturating) with s_in/s_out folded into the horizontal bands; the
output DMA moves int8. Host dequantizes by s_out. This halves DMA bytes
vs fp16io; rel L2 err ~1.5e-2 from the two quantizations (gate 2e-2).
General non-separable 4x4 kernels still work via SVD.
"""

import numpy as np
import concourse.bacc as bacc
import concourse.mybir as mybir
from concourse.tile import TileContext
from concourse.bass_utils import run_bass_kernel_spmd

N_CORES = 8
C, H, W = 128, 256, 256
PRECISION = "i8"  # int8 in+out (~1.5e-2) | "fp16io" (~3e-4) | "fp16" | "fp32r" | "fp32"
# Output quantization scale: s_out = OUT_RANGE * sigma_in * ||k||_F / 127.
# 5.2 covers the output absmax of ~33M-sample gaussian shards with ~0.5%
# headroom (no clipping); a saturation-count retry in kernel() guards the
# general case.
OUT_RANGE = 5.2

_BUILD_CACHE = {}


def _round_f32r(a):
    """Round fp32 array to float32r (11 stored mantissa bits), round-half-up."""
    b = np.ascontiguousarray(a, dtype=np.float32).view(np.uint32)
    b = (b + np.uint32(0x800)) & np.uint32(0xFFFFF000)
    return b.view(np.float32)


def _factorize(kernel4x4):
    """kernel[a,b] = sum_r u_r[a] v_r[b]; returns list of (u, v) float64."""
    k = np.asarray(kernel4x4, dtype=np.float64)
    U, S, Vt = np.linalg.svd(k)
    comps = []
    for r in range(4):
        if S[r] > 1e-9 * max(S[0], 1e-30):
            comps.append((U[:, r] * np.sqrt(S[r]), Vt[r, :] * np.sqrt(S[r])))
    return comps


def _band(taps, n):
    """B[s, s'] = taps[a] where s' = s + a - 1, for a in 0..3, clipped to [0,n)."""
    B = np.zeros((n, n), dtype=np.float64)
    for a in range(4):
        lo = max(0, 1 - a)
        hi = min(n, n + 1 - a)
        s = np.arange(lo, hi)
        B[s, s + a - 1] = taps[a]
    return B


# Live band column ranges per 128-row stationary half (taps span y' in
# [y-1, y+2]): half 0 (rows 0..127) -> cols [0,130); half 1 (rows
# 128..255) -> cols [127,256).
T0_HI = 130
T1_LO = 127

# n1p: channels per group routed through the single-pass 2D conv (one
# PSUM->SBUF copy per element instead of two, at 2x the PE streaming).
# Their output lands transposed [x', y']; the host transposes back.
# Must be < G//ds so each h-half's output DMA still fires from a 2-pass
# channel. 0 = all channels two-pass (pre-mixed behavior).
DEFAULT_CFG = dict(G=16, ds=4, pd=3, cb=1, qin="gpsimd", qout="sync",
                   ycs=0, xin_bufs=2, vt_bufs=6, yout_bufs=2,
                   p1_bufs=4, p2_bufs=4, split=0, n1p=0, ic=2)


def _emit(nc, tc, x, y, bvt, bht, bq, rank, precision, cfg=None):
    cfg = {**DEFAULT_CFG, **(cfg or {})}
    Gc = cfg["G"]
    ds = cfg["ds"]
    cb = cfg["cb"]  # copy batch: j's per PSUM tile / PSUM->SBUF copy
    n1p = cfg["n1p"]
    assert n1p < Gc // ds
    # with 1-pass channels, each channel row is padded to 260 cols (2 zero
    # cols each side) so x-shifted stationary windows never leave the tile
    XP = 260 if n1p else 256
    xo = 2 if n1p else 0
    gsz = Gc // ds
    f32 = mybir.dt.float32
    mmdt = {"fp32": f32, "fp16": mybir.dt.float16, "i8": mybir.dt.float16,
            "fp16io": mybir.dt.float16}.get(precision, mybir.dt.float32r)
    ydt = {"fp16io": mybir.dt.float16, "i8": mybir.dt.int8}.get(precision, f32)
    NG = C // Gc
    # int8 input is cast to fp16 during the DMA -> SWDGE (gpsimd) required
    qin = nc.gpsimd if precision == "i8" else getattr(nc, cfg["qin"])
    qout = getattr(nc, cfg["qout"])
    with (
        tc.tile_pool(name="xin", bufs=cfg["xin_bufs"]) as xin_pool,
        tc.tile_pool(name="vt", bufs=cfg["vt_bufs"]) as vt_pool,
        tc.tile_pool(name="yout", bufs=cfg["yout_bufs"]) as yout_pool,
        tc.tile_pool(name="p1", bufs=cfg["p1_bufs"], space="PSUM") as p1_pool,
        tc.tile_pool(name="p2", bufs=cfg["p2_bufs"], space="PSUM") as p2_pool,
    ):
        pending = []

        def copy_engine(i, which):
            # alternate DVE/ACT; 'which' flips so pass1/pass2 copies of the
            # same batch land on different engines
            if (i + which) % 2 == 0:
                return nc.vector.tensor_copy
            return lambda o, s: nc.scalar.copy(o, s)

        def emit_pass2(p):
            vt, yout, jj, g = p
            p2 = p2_pool.tile([128, 512 * cb], f32, tag="p2")
            for dj in range(cb):
                for q in (0, 1):
                    ops = []
                    for r in range(rank):
                        ops.append((vt[:, dj * 512 + q * 128:
                                       dj * 512 + (q + 1) * 128],
                                    bht[r][0][:, 0:T0_HI], 0, T0_HI))
                        ops.append((vt[:, dj * 512 + 256 + q * 128:
                                       dj * 512 + 256 + (q + 1) * 128],
                                    bht[r][1][:, T1_LO:256], T1_LO, 256))
                    for i, (lhsT, rhs, lo, hi) in enumerate(ops):
                        nc.tensor.matmul(
                            p2[:, dj * 512 + q * 256 + lo:
                               dj * 512 + q * 256 + hi], lhsT, rhs,
                            start=(i == 0), stop=(i == len(ops) - 1),
                        )
            if cfg["ycs"]:
                # fixed engine per q-half so each output DMA chunk waits
                # on exactly one copy engine's stream
                nc.vector.tensor_copy(
                    yout[:, jj:jj + cb, 0:256],
                    p2[:].rearrange("p (j x) -> p j x", x=512)[:, :, 0:256]
                    if cb > 1 else p2[:, 0:256])
                nc.scalar.copy(
                    yout[:, jj:jj + cb, 256:512],
                    p2[:].rearrange("p (j x) -> p j x", x=512)[:, :, 256:512]
                    if cb > 1 else p2[:, 256:512])
            else:
                copy_engine(jj // cb, 1)(yout[:, jj:jj + cb, :], p2[:])
            if (jj + cb) % gsz == 0:
                h = (jj + cb) // gsz - 1
                c0 = g * Gc + h * gsz
                for q in (0, 1):
                    # y DRAM layout is [y', c, x']: contiguous (c,x) runs
                    # of gsz*512 bytes per partition row
                    qout.dma_start(
                        out=y[q * 128:(q + 1) * 128, c0:c0 + gsz, :],
                        in_=yout[:, h * gsz:(h + 1) * gsz,
                                 q * 256:(q + 1) * 256],
                    )

        for g in range(NG):
            xins = []
            for t in (0, 1):
                xt = xin_pool.tile([128, Gc, XP], mmdt,
                                   tag=f"xin{t}", name=f"xin{t}")
                if cfg["split"] and t == 1:
                    # raw int8 load on HWDGE + int8->fp16 convert on gpsimd:
                    # halves this tile's DMA-fabric SBUF writes and keeps
                    # the cast off the (busy) DVE/ACT engines
                    x8t = xin_pool.tile([128, Gc, 256], mybir.dt.int8,
                                        tag=f"x8{t}", name=f"x8{t}")
                    nc.sync.dma_start(
                        out=x8t[:],
                        in_=x[t * 128:(t + 1) * 128, g * Gc:(g + 1) * Gc, :],
                    )
                    nc.gpsimd.tensor_copy(xt[:], x8t[:])
                else:
                    # x DRAM layout is [y, c, x]: contiguous runs per row.
                    # ic>1 splits the load so pass-1 matmuls on the first
                    # channels start before the whole tile has landed and
                    # in/out DMAs interleave at finer grain
                    ic = cfg.get("ic", 1)
                    csz = Gc // ic
                    for k in range(ic):
                        qin.dma_start(
                            out=xt[:, k * csz:(k + 1) * csz, xo:xo + 256],
                            in_=x[t * 128:(t + 1) * 128,
                                  g * Gc + k * csz:g * Gc + (k + 1) * csz, :],
                        )
                if n1p and g < cfg["xin_bufs"]:
                    # zero the pad columns once per pool slot; later DMAs
                    # only write cols 2:258 so the pads stay zero
                    nc.gpsimd.memset(xt[:, :, 0:2], 0.0)
                    nc.gpsimd.memset(xt[:, :, 258:260], 0.0)
                xins.append(xt)
            yout = yout_pool.tile([128, Gc, 512], ydt, tag="yout", name="yout")
            for jj in range(0, Gc, cb):
                if jj < n1p:
                    assert cb == 1
                    # single-pass 2D conv: out^T[x', y'] accumulated from
                    # rank x 4 x-shifts x 2 y-halves stationaries
                    p = p1_pool.tile([128, 512], f32, tag="p1")
                    ops = []
                    for m in (0, 1):
                        for r in range(rank):
                            for b in range(4):
                                for t in (0, 1):
                                    lo, hi = (0, T0_HI) if t == 0 else (T1_LO, 256)
                                    ops.append((
                                        xins[t][:, jj,
                                                m * 128 + 3 - b:
                                                m * 128 + 131 - b],
                                        bq[r][b][t][:, lo:hi],
                                        m * 256 + lo, m * 256 + hi))
                    for i, (lhsT, rhs, o0, o1) in enumerate(ops):
                        nc.tensor.matmul(
                            p[:, o0:o1], lhsT, rhs,
                            start=(i == 0), stop=(i == len(ops) - 1),
                        )
                    copy_engine(jj, 0)(yout[:, jj:jj + 1, :], p[:])
                    continue
                p1 = p1_pool.tile([128, 512 * cb], f32, tag="p1")
                for dj in range(cb):
                    j = jj + dj
                    for m in (0, 1):
                        ops = []
                        for r in range(rank):
                            ops.append((xins[0][:, j, xo + m * 128:
                                                xo + (m + 1) * 128],
                                        bvt[r][0][:, 0:T0_HI], 0, T0_HI))
                            ops.append((xins[1][:, j, xo + m * 128:
                                                xo + (m + 1) * 128],
                                        bvt[r][1][:, T1_LO:256], T1_LO, 256))
                        for i, (lhsT, rhs, lo, hi) in enumerate(ops):
                            nc.tensor.matmul(
                                p1[:, dj * 512 + m * 256 + lo:
                                   dj * 512 + m * 256 + hi], lhsT, rhs,
                                start=(i == 0), stop=(i == len(ops) - 1),
                            )
                vt = vt_pool.tile([128, 512 * cb], mmdt, tag="vt", name="vt")
                copy_engine(jj // cb, 0)(vt[:], p1[:])
                pending.append((vt, yout, jj, g))
                if len(pending) > cfg["pd"]:
                    emit_pass2(pending.pop(0))
        for p in pending:
            emit_pass2(p)


def _build(rank, precision, reps=1, loop_reps=None, cfg=None):
    key = (rank, precision, reps, loop_reps,
           tuple(sorted((cfg or {}).items())))
    if key in _BUILD_CACHE:
        return _BUILD_CACHE[key]
    f32 = mybir.dt.float32
    mmdt = {"fp32": f32, "fp16": mybir.dt.float16, "i8": mybir.dt.float16,
            "fp16io": mybir.dt.float16}.get(precision, mybir.dt.float32r)
    xdt = {"fp32": f32, "i8": mybir.dt.int8}.get(precision, mmdt)
    ydt = {"fp16io": mybir.dt.float16, "i8": mybir.dt.int8}.get(precision, f32)
    n1p = ({**DEFAULT_CFG, **(cfg or {})})["n1p"]
    nc = bacc.Bacc("TRN2", target_bir_lowering=False, debug=False)
    x = nc.dram_tensor("x", [H, C, W], xdt, kind="ExternalInput").ap()
    bv = nc.dram_tensor("bv", [rank, 2, 128, 256], mmdt, kind="ExternalInput").ap()
    bh = nc.dram_tensor("bh", [rank, 2, 128, 256], mmdt, kind="ExternalInput").ap()
    if n1p:
        bqd = nc.dram_tensor("bq", [rank, 4, 2, 128, 256], mmdt,
                             kind="ExternalInput").ap()
    y = nc.dram_tensor("y", [H, C, W], ydt, kind="ExternalOutput").ap()
    with TileContext(nc) as tc:
        with tc.tile_pool(name="bands", bufs=1) as band_pool:
            bvt = [[None, None] for _ in range(rank)]
            bht = [[None, None] for _ in range(rank)]
            bq = [[[None, None] for _ in range(4)] for _ in range(rank)]
            for r in range(rank):
                for t in (0, 1):
                    bvt[r][t] = band_pool.tile([128, 256], mmdt, tag=f"bv{r}{t}", name=f"bv{r}{t}")
                    nc.sync.dma_start(out=bvt[r][t][:], in_=bv[r, t])
                    bht[r][t] = band_pool.tile([128, 256], mmdt, tag=f"bh{r}{t}", name=f"bh{r}{t}")
                    nc.sync.dma_start(out=bht[r][t][:], in_=bh[r, t])
                    if n1p:
                        for b in range(4):
                            bq[r][b][t] = band_pool.tile(
                                [128, 256], mmdt, tag=f"bq{r}{b}{t}",
                                name=f"bq{r}{b}{t}")
                            nc.sync.dma_start(out=bq[r][b][t][:],
                                              in_=bqd[r, b, t])
            if loop_reps is not None:
                with tc.For_i(0, loop_reps, 1):
                    _emit(nc, tc, x, y, bvt, bht, bq, rank, precision, cfg)
            else:
                for _ in range(reps):
                    _emit(nc, tc, x, y, bvt, bht, bq, rank, precision, cfg)
    nc.compile()
    _BUILD_CACHE[key] = nc
    return nc


def _prep_inputs(fmap, kernel4x4, precision, s_out_boost=1.0, n1p=None):
    if n1p is None:
        n1p = DEFAULT_CFG["n1p"]
    comps = _factorize(kernel4x4)
    rank = max(1, len(comps))
    while len(comps) < rank:
        comps.append((np.zeros(4), np.zeros(4)))

    def bands(comps_i):
        bv = np.zeros((rank, 2, 128, 256), dtype=np.float32)
        bh = np.zeros((rank, 2, 128, 256), dtype=np.float32)
        bq = np.zeros((rank, 4, 2, 128, 256), dtype=np.float32)
        for r, (u, v) in enumerate(comps_i):
            Bv = _band(u, H).astype(np.float32).reshape(2, 128, 256)
            bv[r] = Bv
            bh[r] = _band(v, W).astype(np.float32).reshape(2, 128, 256)
            for b in range(4):
                bq[r, b] = np.float32(v[b]) * Bv
        if precision == "fp32r":
            return _round_f32r(bv), _round_f32r(bh), _round_f32r(bq)
        if precision in ("fp16", "fp16io", "i8"):
            return (bv.astype(np.float16), bh.astype(np.float16),
                    bq.astype(np.float16))
        return bv, bh, bq

    knorm = float(np.sqrt(np.square(np.asarray(kernel4x4, np.float64)).sum()))
    in_maps, s_outs = [], []
    if precision != "i8":
        bv, bh, bq = bands(comps)
    for i in range(N_CORES):
        shard = np.ascontiguousarray(fmap[i].transpose(1, 0, 2),
                                     dtype=np.float32)  # [y, c, x]
        if precision == "fp32r":
            shard = _round_f32r(shard)
        elif precision in ("fp16", "fp16io"):
            shard = shard.astype(np.float16)
        elif precision == "i8":
            s_in = float(np.abs(shard).max()) / 127.0
            s_out = OUT_RANGE * float(shard.std()) * knorm / 127.0 * s_out_boost
            s_outs.append(s_out)
            alpha = s_in / s_out
            bv, bh, bq = bands([(u, v * alpha) for (u, v) in comps])
            shard = np.clip(np.rint(shard * (1.0 / s_in)),
                            -127, 127).astype(np.int8)
        m = {"x": shard, "bv": bv, "bh": bh}
        if n1p:
            m["bq"] = bq
        in_maps.append(m)
    return rank, in_maps, s_outs


def _run(nc, in_maps):
    last_err = None
    for _attempt in range(3):
        try:
            return run_bass_kernel_spmd(nc, in_maps, list(range(N_CORES)),
                                        trace=False)
        except Exception as e:  # transient device wedge -> retry
            last_err = e
            import time
            time.sleep(2.0)
    raise last_err


def kernel(fmap, kernel):
    fmap = np.asarray(fmap)
    kern = np.asarray(kernel)
    assert fmap.shape == (N_CORES, C, H, W), fmap.shape
    boost = 1.0
    rank, in_maps, s_outs = _prep_inputs(fmap, kern, PRECISION)
    nc = _build(rank, PRECISION)
    res = _run(nc, in_maps)
    if PRECISION == "i8":
        # s_out underestimated the output range -> saturation; retry coarser
        for _ in range(3):
            n_sat = sum(int((res.results[i]["y"] == 127).sum() +
                            (res.results[i]["y"] == -128).sum())
                        for i in range(N_CORES))
            if n_sat <= 4096:
                break
            boost *= 1.5
            rank, in_maps, s_outs = _prep_inputs(fmap, kern, PRECISION,
                                                 s_out_boost=boost)
            res = _run(nc, in_maps)
    out = np.stack([res.results[i]["y"].transpose(1, 0, 2)
                    for i in range(N_CORES)], axis=0)
    n1p, Gc = DEFAULT_CFG["n1p"], DEFAULT_CFG["G"]
    if n1p:
        # single-pass channels come back transposed [x', y']
        idx = [j for j in range(C) if j % Gc < n1p]
        out[:, idx] = np.swapaxes(out[:, idx], 2, 3)
    out = out.astype(np.float32)
    if PRECISION == "i8":
        out *= np.asarray(s_outs, np.float32)[:, None, None, None]
    return np.ascontiguousarray(out)

